# revision 1
# baseline (speedup 1.0000x reference)
# Multi-head attention with RoPE, tensor-parallel over heads on 8 NeuronCores.
#
# Problem: B=2, N=2048, D=1024, H=16 heads, head_dim=64.
#   q/k/v = x @ W{q,k,v}.T + b;  RoPE(q), RoPE(k);  softmax(q k^T / 8) v;
#   out = attn @ Wo.T + bo.
#
# Sharding: 2 heads per core (column-parallel QKV, row-parallel out-proj).
# Each core emits a partial [T, D] output (bf16); host sums the 8 partials
# and adds bo.
#
# Per-core layout strategy (all matmul inputs bf16, fp32 accumulation):
#   xT   [D, T]    : x transposed on host, so the contraction dim d is on
#                    partitions for every projection matmul.
#   q,k  [E=128, T]: "transposed" activations (2 heads * 64 on partitions).
#                    Biases folded in via tensor_scalar_add during the
#                    PSUM->SBUF evacuation (per-partition scalar).
#   RoPE           : rotate-half done as a 128x128 constant permutation matmul
#                    (rot = P @ q), then q'' = q*cos + rot*sin on DVE.
#   v              : projected as vT [E, T] (tokens moving, N=512 per matmul),
#                    bias folded on evacuation, then PE-transposed per
#                    128-token chunk into v_sb [tk, 130] = [1 | v_h0 | v_h1 | 1]
#                    so attn@v also yields softmax row sums (ones columns).
#   scores^T       : [tk, tq] via lhsT=k (so exp output feeds attn@v directly
#                    as the moving operand, free dim 512).
#   attn@v         : oT[65, tq] = [1|v_h]^T @ expT, accumulated over tk in
#                    PSUM; h0: row 0 = denom, rows 1-64 = out; h1: rows 0-63 =
#                    out, row 64 = denom.
#   normalize      : reciprocal_approx_fast of the denom row, broadcast to 128
#                    partitions via a K=1 matmul, multiplied into the stacked
#                    on_sb [128, b, tq] during the PSUM->SBUF copy.
#   out-proj       : y[t, :] = on^T @ woT per 128-token tile (single K=128
#                    matmul per 512-col chunk; both heads contracted at once).
#                    bo is added by the host after the partial-sum gather.

import numpy as np
import ml_dtypes

import concourse.bass as bass
import concourse.mybir as mybir
import concourse.tile as tile
from concourse import bacc

B, N, D, H = 2, 2048, 1024, 16
HD = 64
T = B * N                 # 4096 tokens
NCORES = 8
HPC = H // NCORES         # 2 heads per core
E = HPC * HD              # 128 per-core projection columns
KD = D // 128             # 8 contraction tiles for d
ROPE_BASE = 10000.0

BF = mybir.dt.bfloat16
F32 = mybir.dt.float32
F32R = mybir.dt.float32r

TQC = 1024                # tq chunk (exp granularity / psum width)
NTQC = N // TQC           # 2 per batch
NKC = N // 128            # 16 key chunks per batch


def build_nc():
    nc = bacc.Bacc(trn_type="TRN2", target_bir_lowering=False, debug=False)

    xT = nc.dram_tensor("xT", [D, T], BF, kind="ExternalInput").ap()
    wqT = nc.dram_tensor("wqT", [D, E], BF, kind="ExternalInput").ap()
    wkT = nc.dram_tensor("wkT", [D, E], BF, kind="ExternalInput").ap()
    wvT = nc.dram_tensor("wvT", [D, E], BF, kind="ExternalInput").ap()
    woT = nc.dram_tensor("woT", [E, D], BF, kind="ExternalInput").ap()
    bcol = nc.dram_tensor("bcol", [E, 3], F32, kind="ExternalInput").ap()
    cosb = nc.dram_tensor("cosb", [E, N], BF, kind="ExternalInput").ap()
    sinb = nc.dram_tensor("sinb", [E, N], BF, kind="ExternalInput").ap()
    rotT = nc.dram_tensor("rotT", [E, E], BF, kind="ExternalInput").ap()
    idT = nc.dram_tensor("idT", [E, E], BF, kind="ExternalInput").ap()
    ones1 = nc.dram_tensor("ones1", [1, TQC], F32R, kind="ExternalInput").ap()
    y = nc.dram_tensor("y", [T, D], BF, kind="ExternalOutput").ap()

    with tile.TileContext(nc) as tc:
        _build(tc, nc, xT, wqT, wkT, wvT, woT, bcol, cosb, sinb,
               rotT, idT, ones1, y)
    nc.compile()  # bacc legalization: splits multi-wait instructions etc.
    return nc


def _build(tc, nc, xT, wqT, wkT, wvT, woT, bcol, cosb, sinb,
           rotT, idT, ones1, y):
    with (
        tc.tile_pool(name="consts", bufs=1) as consts,
        tc.tile_pool(name="xbig", bufs=1) as xbig,
        tc.tile_pool(name="acts", bufs=1) as acts,
        tc.tile_pool(name="small", bufs=3) as small,
    ):
        # ---- constants / weights ----
        wq_sb = consts.tile([128, KD, E], BF, tag="wq")
        wk_sb = consts.tile([128, KD, E], BF, tag="wk")
        wv_sb = consts.tile([128, KD, E], BF, tag="wv")
        nc.sync.dma_start(out=wq_sb, in_=wqT.rearrange("(k p) e -> p k e", p=128))
        x_sb = xbig.tile([128, KD, T], BF, tag="big")
        xTr = xT.rearrange("(k p) t -> p k t", p=128)
        nc.sync.dma_start(out=x_sb[:, :, 0:512], in_=xTr[:, :, 0:512])
        nc.sync.dma_start(out=wk_sb, in_=wkT.rearrange("(k p) e -> p k e", p=128))
        nc.sync.dma_start(out=wv_sb, in_=wvT.rearrange("(k p) e -> p k e", p=128))
        nc.sync.dma_start(out=x_sb[:, :, 512:1024], in_=xTr[:, :, 512:1024])
        wo_sb = consts.tile([E, D], BF, tag="wo")
        nc.sync.dma_start(out=wo_sb, in_=woT)
        bcol_sb = consts.tile([E, 3], F32, tag="bcol")
        nc.sync.dma_start(out=bcol_sb, in_=bcol)
        cos_sb = consts.tile([E, N], BF, tag="cos")
        sin_sb = consts.tile([E, N], BF, tag="sin")
        nc.sync.dma_start(out=cos_sb, in_=cosb)
        nc.sync.dma_start(out=sin_sb, in_=sinb)
        rot_sb = consts.tile([E, E], BF, tag="rot")
        nc.sync.dma_start(out=rot_sb, in_=rotT)
        id_sb = consts.tile([E, E], BF, tag="idT")
        nc.sync.dma_start(out=id_sb, in_=idT)
        ones1_sb = consts.tile([1, TQC], F32R, tag="ones1")
        nc.sync.dma_start(out=ones1_sb, in_=ones1)

        # ---- rest of x^T (slices 0/1 were DMA'd early above) ----
        for ci in range(2, T // 512):
            nc.sync.dma_start(out=x_sb[:, :, ci * 512:(ci + 1) * 512],
                              in_=xTr[:, :, ci * 512:(ci + 1) * 512])

        # ---- persistent activations ----
        q_sb = acts.tile([E, T], BF, tag="q_sb")
        k_sb = acts.tile([E, T], BF, tag="k_sb")
        q2 = acts.tile([E, T], BF, tag="q2")
        k2 = acts.tile([E, T], BF, tag="k2")
        # v tiles: [tk 128, tk-chunk 32, head 2, 64+ones] = [v_h | 1] per head
        v_sb = acts.tile([128, T // 128, HPC, HD + 1], BF, tag="v_sb")
        # normalized attention output, stacked: [e 128, b 2, tq 2048]
        on_sb = acts.tile([E, B, N], BF, tag="on_sb")

        nc.vector.memset(v_sb[:, :, :, HD:HD + 1], 1.0)

        # ================= phase 1: projections + rope =================
        with (
            tc.tile_pool(name="ps_qk", bufs=2, space="PSUM") as ps_qk,
            tc.tile_pool(name="ps_v", bufs=2, space="PSUM") as ps_v,
            tc.tile_pool(name="ps_r", bufs=2, space="PSUM") as ps_r,
            tc.tile_pool(name="ps_t", bufs=2, space="PSUM") as ps_t,
        ):
            # Emission interleaved per 512-token slice so compute tracks the
            # incoming x DMA stream and the PE never sits idle long.
            for ci in range(T // 512):
                sl = slice(ci * 512, (ci + 1) * 512)
                npos = (ci * 512) % N
                tsl = slice(npos, npos + 512)
                # q / k projection + rope for this slice
                for dst, w, bc, dst2 in ((q_sb, wq_sb, 0, q2),
                                         (k_sb, wk_sb, 1, k2)):
                    ps = ps_qk.tile([128, 512], F32, tag="ps_qk")
                    for k in range(KD):
                        nc.tensor.matmul(
                            ps, w[:, k, :], x_sb[:, k, sl],
                            start=(k == 0), stop=(k == KD - 1))
                    nc.vector.tensor_scalar_add(
                        dst[:, sl], ps, bcol_sb[:, bc:bc + 1])
                    # rope: dst2 = dst*cos + (P@dst)*sin
                    psr = ps_r.tile([128, 512], F32, tag="ps_r")
                    nc.tensor.matmul(psr, rot_sb, dst[:, sl],
                                     start=True, stop=True)
                    t1 = small.tile([128, 512], BF, tag="rope_t1")
                    nc.vector.tensor_mul(t1, dst[:, sl], cos_sb[:, tsl])
                    t2 = small.tile([128, 512], BF, tag="rope_t2")
                    nc.vector.tensor_mul(t2, psr, sin_sb[:, tsl])
                    nc.vector.tensor_add(dst2[:, sl], t1, t2)
                # vT projection for this slice, then transpose per 128 tokens
                psv = ps_v.tile([128, 512], F32, tag="ps_v")
                for k in range(KD):
                    nc.tensor.matmul(
                        psv, wv_sb[:, k, :], x_sb[:, k, sl],
                        start=(k == 0), stop=(k == KD - 1))
                vts = small.tile([128, 512], BF, tag="vts")
                nc.vector.tensor_scalar_add(vts, psv, bcol_sb[:, 2:3])
                for s in range(4):
                    cv = ci * 4 + s
                    pst = ps_t.tile([128, 128], BF, tag="ps_t")
                    nc.tensor.transpose(
                        pst, vts[:, s * 128:(s + 1) * 128], id_sb)
                    # strided write: head h's 64 v-columns at [cv, h, 0:64]
                    nc.vector.tensor_copy(v_sb[:, cv, :, 0:HD], pst)

        # ========= phase 2+3: attention + output projection =========
        # Per (b, tqc): for each key chunk tkc, scores (both heads,
        # row-packed) -> exp -> attn@v MMs, interleaved so the PE always has
        # attn work for chunk tkc while ACT computes exp for chunk tkc+1.
        with (
            tc.tile_pool(name="ps_sc", bufs=2, space="PSUM") as ps_sc,
            tc.tile_pool(name="ps_o", bufs=2, space="PSUM") as ps_o,
        ):
            # Deferred-work queue: each block's normalization chain and the
            # per-batch output-projection chunks are emitted one-per-tkc
            # inside LATER blocks' loops, so the scores/exp/attn drum never
            # pauses.
            pending = []
            _state = {"drain": False}

            def norm_item(b, tqc, h, ou2, rs):
                # ou2: stacked unnormalized attn out [128, TQC] bf16 (SBUF);
                #      this head's rows live at partitions h*64..h*64+63
                # rs: softmax denominators [1, TQC] f32 (SBUF)
                def emit():
                    rc = small.tile([1, TQC], F32, tag="recip", bufs=2,
                                    name=f"rc_{b}_{tqc}_{h}")
                    nc.vector.reciprocal_approx_fast(out=rc, in_=rs)
                    rbs = small.tile([128, TQC], F32, tag="recipb", bufs=2,
                                      name=f"rbs_{b}_{tqc}_{h}")
                    hsl = slice(h * HD, (h + 1) * HD)
                    for nn in range(TQC // 512):
                        csl = slice(nn * 512, (nn + 1) * 512)
                        nc.gpsimd.partition_broadcast(
                            out_ap=rbs[:, csl], in_ap=rc[:, csl], channels=128)
                        nc.vector.tensor_mul(
                            on_sb[hsl, b, tqc * TQC + nn * 512:
                                  tqc * TQC + (nn + 1) * 512],
                            ou2[hsl, csl], rbs[hsl, csl])
                    if h == HPC - 1:
                        # both heads of (b, tqc) normalized -> the matching
                        # output-projection chunks are now eligible
                        for ci in range(tqc * 8, tqc * 8 + 8):
                            pending.append(y_item(b, ci))
                return emit

            def y_item(b, ci):
                def emit():
                    psy = ps_sc.tile([128, D], F32, tag="ps_sc",
                                     name=f"psy_{b}_{ci}")
                    for eo in range(D // 512):
                        nc.tensor.matmul(
                            psy[:, eo * 512:(eo + 1) * 512],
                            on_sb[:, b, ci * 128:(ci + 1) * 128],
                            wo_sb[:, eo * 512:(eo + 1) * 512],
                            start=True, stop=True)
                    ysb = small.tile([128, D], BF, tag="ysb", bufs=4,
                                     name=f"ysb_{b}_{ci}")
                    if _state["drain"] and ci % 2 == 1:
                        nc.scalar.copy(ysb, psy)
                    else:
                        nc.vector.tensor_copy(ysb, psy)
                    nc.sync.dma_start(
                        out=y[b * N + ci * 128: b * N + (ci + 1) * 128, :],
                        in_=ysb)
                return emit

            for b in range(B):
                for tqc in range(NTQC):
                    tq0 = b * N + tqc * TQC  # global tq base
                    exp_t = xbig.tile([128, HPC, NKC, TQC], BF, tag="big")
                    exp0 = small.tile([128, HPC, TQC], BF, tag="exp0",
                                      bufs=2, name=f"exp0_{b}_{tqc}")
                    ots = [ps_o.tile([HD + 1, TQC], F32, tag="ps_o",
                                     name=f"ot_{b}_{tqc}_{h}")
                           for h in range(HPC)]

                    def attn_mms(j):
                        # attn@v for key chunk j (consumes exp of chunk j)
                        for h in range(HPC):
                            vt = v_sb[:, b * NKC + j, h, :]
                            src = exp0[:, h, :] if j == 0 else exp_t[:, h, j, :]
                            for nn in range(TQC // 512):
                                nc.tensor.matmul(
                                    ots[h][:, nn * 512:(nn + 1) * 512], vt,
                                    src[:, nn * 512:(nn + 1) * 512],
                                    start=(j == 0), stop=(j == NKC - 1))

                    # Software-pipelined: the PE's attn@v for chunk tkc-1 is
                    # emitted after exp(tkc) so the PE never waits on the
                    # same-iteration exp; ACT (exp) is the steady-state drum.
                    # One deferred item (prev block's normalization / y-proj
                    # chunk) is woven in per tkc iteration.
                    for tkc in range(NKC):
                        scs = []
                        for h in range(HPC):
                            sc = ps_sc.tile([128, TQC], F32, tag="ps_sc")
                            lhsT = k2[h * HD:(h + 1) * HD,
                                      b * N + tkc * 128: b * N + (tkc + 1) * 128]
                            for nn in range(TQC // 512):
                                nc.tensor.matmul(
                                    sc[:, nn * 512:(nn + 1) * 512], lhsT,
                                    q2[h * HD:(h + 1) * HD,
                                       tq0 + nn * 512: tq0 + (nn + 1) * 512],
                                    start=True, stop=True)
                            scs.append(sc)
                        for h in range(HPC):
                            eout = (exp0[:, h, :] if tkc == 0
                                    else exp_t[:, h, tkc, :])
                            nc.scalar.activation(
                                out=eout, in_=scs[h],
                                func=mybir.ActivationFunctionType.Exp,
                                scale=float(HD) ** -0.5)
                        if tkc > 0:
                            attn_mms(tkc - 1)
                        if tkc >= 1 and pending:
                            pending.pop(0)()
                    attn_mms(NKC - 1)

                    # Evacuate the ot PSUM tiles quickly so the next block's
                    # attn matmuls get the slots; the reciprocal chain is
                    # deferred via `pending`.
                    # Each head: psum rows 0..63 = out, row 64 = denom.
                    ou2 = small.tile([128, TQC], BF, tag="ou", bufs=2,
                                     name=f"ou_{b}_{tqc}")
                    nc.vector.tensor_copy(ou2[0:HD, :], ots[0][0:HD, :])
                    nc.scalar.copy(ou2[HD:2 * HD, :], ots[1][0:HD, :])
                    rss = []
                    for h in range(HPC):
                        rs = small.tile([1, TQC], F32, tag="rs", bufs=4,
                                        name=f"rs_{b}_{tqc}_{h}")
                        if h == 0:
                            nc.vector.tensor_copy(rs, ots[h][HD:HD + 1, :])
                        else:
                            nc.scalar.copy(rs, ots[h][HD:HD + 1, :])
                        rss.append(rs)
                    for h in range(HPC):
                        pending.append(norm_item(b, tqc, h, ou2, rss[h]))

            # drain remaining deferred work (last block's norms + final ys)
            _state["drain"] = True
            while pending:
                pending.pop(0)()


def _host_inputs(x, Wq, Wk, Wv, Wo, bq, bk, bv, bo):
    """Build the 8 per-core input maps (host-side sharding + layout prep)."""
    bf16 = ml_dtypes.bfloat16
    xTh = np.ascontiguousarray(x.reshape(T, D).T).astype(bf16)

    # rope tables: row e uses freq (e % 64) % 32; positions along columns
    i = (np.arange(E) % HD) % (HD // 2)
    inv_freq = ROPE_BASE ** (-2.0 * i / HD)  # [E]
    ang = np.arange(N)[None, :] * inv_freq[:, None]          # [E, N]
    cosb = np.cos(ang).astype(bf16)
    sinb = np.sin(ang).astype(bf16)

    # rotate-half permutation: rot = P @ q (per 64-block)
    P = np.zeros((E, E), dtype=np.float32)
    for h in range(HPC):
        for j in range(HD // 2):
            P[h * HD + j, h * HD + j + HD // 2] = -1.0
            P[h * HD + j + HD // 2, h * HD + j] = 1.0
    rotT = np.ascontiguousarray(P.T).astype(bf16)
    idT = np.eye(E, dtype=np.float32).astype(bf16)

    ones1 = np.ones((1, TQC), dtype=np.float32)

    in_maps = []
    for c in range(NCORES):
        sl = slice(c * E, (c + 1) * E)
        in_maps.append({
            "xT": xTh,
            "wqT": np.ascontiguousarray(Wq[sl, :].T).astype(bf16),
            "wkT": np.ascontiguousarray(Wk[sl, :].T).astype(bf16),
            "wvT": np.ascontiguousarray(Wv[sl, :].T).astype(bf16),
            "woT": np.ascontiguousarray(Wo[:, sl].T).astype(bf16),
            "bcol": np.stack([bq[sl], bk[sl], bv[sl]], axis=1)
                .astype(np.float32),
            "cosb": cosb,
            "sinb": sinb,
            "rotT": rotT,
            "idT": idT,
            "ones1": ones1,
        })
    return in_maps


_NC = None


def kernel(x, Wq, Wk, Wv, Wo, bq, bk, bv, bo):
    from concourse.bass_utils import run_bass_kernel_spmd

    global _NC
    if _NC is None:
        _NC = build_nc()
    bo = np.asarray(bo, dtype=np.float32)
    in_maps = _host_inputs(np.asarray(x, dtype=np.float32),
                           np.asarray(Wq, dtype=np.float32),
                           np.asarray(Wk, dtype=np.float32),
                           np.asarray(Wv, dtype=np.float32),
                           np.asarray(Wo, dtype=np.float32),
                           np.asarray(bq, dtype=np.float32),
                           np.asarray(bk, dtype=np.float32),
                           np.asarray(bv, dtype=np.float32),
                           bo)
    res = run_bass_kernel_spmd(_NC, in_maps, core_ids=list(range(NCORES)))
    out = np.zeros((T, D), dtype=np.float32)
    for r in res.results:
        out += np.asarray(r["y"], dtype=np.float32)
    out += bo[None, :]
    return out.reshape(B, N, D)



# revision 10
# speedup vs baseline: 1.0027x; 1.0027x over previous
# Multi-head attention with RoPE, tensor-parallel over heads on 8 NeuronCores.
#
# Problem: B=2, N=2048, D=1024, H=16 heads, head_dim=64.
#   q/k/v = x @ W{q,k,v}.T + b;  RoPE(q), RoPE(k);  softmax(q k^T / 8) v;
#   out = attn @ Wo.T + bo.
#
# Sharding: 2 heads per core (column-parallel QKV, row-parallel out-proj).
# Each core emits a partial [T, D] output (f32); host sums the 8 partials
# and adds bo.
#
# Per-core layout strategy (all matmul inputs bf16, fp32 accumulation):
#   xT   [D, T]    : x transposed on host, so the contraction dim d is on
#                    partitions for every projection matmul.
#   q,k  [E=128, T]: "transposed" activations (2 heads * 64 on partitions).
#                    Biases folded in via tensor_scalar_add during the
#                    PSUM->SBUF evacuation (per-partition scalar).
#   RoPE           : rotate-half done as a 128x128 constant permutation matmul
#                    (rot = P @ q), then q'' = q*cos + rot*sin on DVE.
#   v              : projected as vT [E, T] (tokens moving, N=512 per matmul),
#                    bias folded on evacuation, then PE-transposed per
#                    128-token chunk into v_sb [tk, 130] = [1 | v_h0 | v_h1 | 1]
#                    so attn@v also yields softmax row sums (ones columns).
#   scores^T       : [tk, tq] via lhsT=k (so exp output feeds attn@v directly
#                    as the moving operand, free dim 512). The two heads'
#                    matmuls are emitted interleaved so they run CONCURRENTLY
#                    on disjoint PE row-groups (h0 contracts partitions 0-63 ->
#                    tile (0,0); h1 partitions 64-127 -> tile (64,0)).
#   attn@v         : oT[65, tq] = [v_h|1]^T @ expT, accumulated over tk in
#                    PSUM; per head rows 0-63 = out, row 64 = denom.
#   evacuation     : ot PSUM tiles drained by DMA (f32, partition-shifting),
#                    keeping ACT free for exp. Normalization = one DVE mul
#                    with a gpsimd-broadcast reciprocal tile.
#   out-proj       : y[t, :] = on^T @ woT per 128-token tile (single K=128
#                    matmul per 512-col chunk; both heads contracted at once).
#                    PSUM result DMA'd straight to DRAM as f32; host sums the
#                    8 partial outputs and adds bo.

import numpy as np
import ml_dtypes

import concourse.bass as bass
import concourse.mybir as mybir
import concourse.tile as tile
from concourse import bacc

B, N, D, H = 2, 2048, 1024, 16
HD = 64
T = B * N                 # 4096 tokens
NCORES = 8
HPC = H // NCORES         # 2 heads per core
E = HPC * HD              # 128 per-core projection columns
KD = D // 128             # 8 contraction tiles for d
ROPE_BASE = 10000.0

BF = mybir.dt.bfloat16
F32 = mybir.dt.float32

TQC = 1024                # tq chunk (exp granularity / psum width)
NTQC = N // TQC           # 2 per batch
NKC = N // 128            # 16 key chunks per batch


def build_nc():
    nc = bacc.Bacc(trn_type="TRN2", target_bir_lowering=False, debug=False)

    xT = nc.dram_tensor("xT", [D, T], BF, kind="ExternalInput").ap()
    wqT = nc.dram_tensor("wqT", [D, E], BF, kind="ExternalInput").ap()
    wkT = nc.dram_tensor("wkT", [D, E], BF, kind="ExternalInput").ap()
    wvT = nc.dram_tensor("wvT", [D, E], BF, kind="ExternalInput").ap()
    woT = nc.dram_tensor("woT", [E, D], BF, kind="ExternalInput").ap()
    bcol = nc.dram_tensor("bcol", [E, 3], F32, kind="ExternalInput").ap()
    cosb = nc.dram_tensor("cosb", [E, N], BF, kind="ExternalInput").ap()
    sinb = nc.dram_tensor("sinb", [E, N], BF, kind="ExternalInput").ap()
    rotT = nc.dram_tensor("rotT", [E, E], BF, kind="ExternalInput").ap()
    idT = nc.dram_tensor("idT", [E, E], BF, kind="ExternalInput").ap()
    y = nc.dram_tensor("y", [T, D], BF, kind="ExternalOutput").ap()

    with tile.TileContext(nc) as tc:
        _build(tc, nc, xT, wqT, wkT, wvT, woT, bcol, cosb, sinb,
               rotT, idT, y)
    nc.compile()  # bacc legalization: splits multi-wait instructions etc.
    return nc


def _build(tc, nc, xT, wqT, wkT, wvT, woT, bcol, cosb, sinb,
           rotT, idT, y):
    with (
        tc.tile_pool(name="consts", bufs=1) as consts,
        tc.tile_pool(name="xbig", bufs=1) as xbig,
        tc.tile_pool(name="acts", bufs=1) as acts,
        tc.tile_pool(name="small", bufs=3) as small,
    ):
        # ---- constants / weights ----
        wq_sb = consts.tile([128, KD, E], BF, tag="wq")
        wk_sb = consts.tile([128, KD, E], BF, tag="wk")
        wv_sb = consts.tile([128, KD, E], BF, tag="wv")
        nc.sync.dma_start(out=wq_sb, in_=wqT.rearrange("(k p) e -> p k e", p=128))
        x_sb = xbig.tile([128, KD, T], BF, tag="big")
        xTr = xT.rearrange("(k p) t -> p k t", p=128)
        nc.sync.dma_start(out=x_sb[:, :, 0:512], in_=xTr[:, :, 0:512])
        nc.sync.dma_start(out=wk_sb, in_=wkT.rearrange("(k p) e -> p k e", p=128))
        nc.sync.dma_start(out=wv_sb, in_=wvT.rearrange("(k p) e -> p k e", p=128))
        nc.sync.dma_start(out=x_sb[:, :, 512:1024], in_=xTr[:, :, 512:1024])
        wo_sb = consts.tile([E, D], BF, tag="wo")
        nc.sync.dma_start(out=wo_sb, in_=woT)
        bcol_sb = consts.tile([E, 3], F32, tag="bcol")
        nc.sync.dma_start(out=bcol_sb, in_=bcol)
        cos_sb = consts.tile([E, N], BF, tag="cos")
        sin_sb = consts.tile([E, N], BF, tag="sin")
        nc.sync.dma_start(out=cos_sb, in_=cosb)
        nc.sync.dma_start(out=sin_sb, in_=sinb)
        rot_sb = consts.tile([E, E], BF, tag="rot")
        nc.sync.dma_start(out=rot_sb, in_=rotT)
        id_sb = consts.tile([E, E], BF, tag="idT")
        nc.sync.dma_start(out=id_sb, in_=idT)

        # ---- rest of x^T (slices 0/1 were DMA'd early above) ----
        for ci in range(2, T // 512):
            nc.sync.dma_start(out=x_sb[:, :, ci * 512:(ci + 1) * 512],
                              in_=xTr[:, :, ci * 512:(ci + 1) * 512])

        # ---- persistent activations ----
        q_sb = acts.tile([E, T], BF, tag="q_sb")
        k_sb = acts.tile([E, T], BF, tag="k_sb")
        q2 = acts.tile([E, T], BF, tag="q2")
        k2 = acts.tile([E, T], BF, tag="k2")
        # v tiles: [tk 128, tk-chunk 32, head 2, 64+ones] = [v_h | 1] per head
        v_sb = acts.tile([128, T // 128, HPC, HD + 1], BF, tag="v_sb")
        # normalized attention output, stacked: [e 128, b 2, tq 2048]
        on_sb = acts.tile([E, B, N], BF, tag="on_sb")

        nc.vector.memset(v_sb[:, :, :, HD:HD + 1], 1.0)

        # ================= phase 1: projections + rope =================
        with (
            tc.tile_pool(name="ps_qk", bufs=2, space="PSUM") as ps_qk,
            tc.tile_pool(name="ps_v", bufs=2, space="PSUM") as ps_v,
            tc.tile_pool(name="ps_r", bufs=2, space="PSUM") as ps_r,
            tc.tile_pool(name="ps_t", bufs=2, space="PSUM") as ps_t,
        ):
            # Emission interleaved per 512-token slice so compute tracks the
            # incoming x DMA stream and the PE never sits idle long.
            for ci in range(T // 512):
                sl = slice(ci * 512, (ci + 1) * 512)
                npos = (ci * 512) % N
                tsl = slice(npos, npos + 512)
                # q / k projection + rope for this slice
                for dst, w, bc, dst2 in ((q_sb, wq_sb, 0, q2),
                                         (k_sb, wk_sb, 1, k2)):
                    ps = ps_qk.tile([128, 512], F32, tag="ps_qk")
                    for k in range(KD):
                        nc.tensor.matmul(
                            ps, w[:, k, :], x_sb[:, k, sl],
                            start=(k == 0), stop=(k == KD - 1))
                    nc.vector.tensor_scalar_add(
                        dst[:, sl], ps, bcol_sb[:, bc:bc + 1])
                    # rope: dst2 = dst*cos + (P@dst)*sin
                    psr = ps_r.tile([128, 512], F32, tag="ps_r")
                    nc.tensor.matmul(psr, rot_sb, dst[:, sl],
                                     start=True, stop=True)
                    t1 = small.tile([128, 512], BF, tag="rope_t1")
                    nc.vector.tensor_mul(t1, dst[:, sl], cos_sb[:, tsl])
                    t2 = small.tile([128, 512], BF, tag="rope_t2")
                    nc.vector.tensor_mul(t2, psr, sin_sb[:, tsl])
                    nc.vector.tensor_add(dst2[:, sl], t1, t2)
                # vT projection for this slice, then transpose per 128 tokens
                psv = ps_v.tile([128, 512], F32, tag="ps_v")
                for k in range(KD):
                    nc.tensor.matmul(
                        psv, wv_sb[:, k, :], x_sb[:, k, sl],
                        start=(k == 0), stop=(k == KD - 1))
                vts = small.tile([128, 512], BF, tag="vts")
                nc.vector.tensor_scalar_add(vts, psv, bcol_sb[:, 2:3])
                for s in range(4):
                    cv = ci * 4 + s
                    pst = ps_t.tile([128, 128], BF, tag="ps_t")
                    nc.tensor.transpose(
                        pst, vts[:, s * 128:(s + 1) * 128], id_sb)
                    # strided write: head h's 64 v-columns at [cv, h, 0:64]
                    nc.vector.tensor_copy(v_sb[:, cv, :, 0:HD], pst)

        # ========= phase 2+3: attention + output projection =========
        # Per (b, tqc): for each key chunk tkc, scores (both heads,
        # row-packed) -> exp -> attn@v MMs, interleaved so the PE always has
        # attn work for chunk tkc while ACT computes exp for chunk tkc+1.
        with (
            tc.tile_pool(name="ps_sc", bufs=2, space="PSUM") as ps_sc,
            tc.tile_pool(name="ps_o", bufs=2, space="PSUM") as ps_o,
        ):
            # Deferred-work queue: each block's normalization chain and the
            # per-batch output-projection chunks are emitted one-per-tkc
            # inside LATER blocks' loops, so the scores/exp/attn drum never
            # pauses.
            pending = []
            _state = {"drain": False}

            def norm_item(b, tqc, ou2f, rss):
                # ou2f: stacked unnormalized attn out [128, TQC] f32 (SBUF)
                # rss: softmax denominators [1, TQC] f32 (SBUF) per head
                def emit():
                    for h in range(HPC):
                        rc = small.tile([1, TQC], F32, tag="recip", bufs=2,
                                        name=f"rc_{b}_{tqc}_{h}")
                        nc.vector.reciprocal_approx_fast(out=rc, in_=rss[h])
                        rbs = small.tile([128, TQC], F32, tag="recipb",
                                         bufs=2, name=f"rbs_{b}_{tqc}_{h}")
                        nc.gpsimd.partition_broadcast(
                            out_ap=rbs, in_ap=rc, channels=128)
                        hsl = slice(h * HD, (h + 1) * HD)
                        nc.vector.tensor_mul(
                            on_sb[hsl, b, tqc * TQC:(tqc + 1) * TQC],
                            ou2f[hsl, :], rbs[hsl, :])
                    # both heads of (b, tqc) normalized -> the matching
                    # output-projection chunks are now eligible
                    for ci in range(tqc * 8, tqc * 8 + 8):
                        pending.append(y_item(b, ci))
                return emit

            def y_item(b, ci):
                def emit():
                    psy = ps_sc.tile([128, D], F32, tag="ps_sc",
                                     name=f"psy_{b}_{ci}")
                    for eo in range(D // 512):
                        nc.tensor.matmul(
                            psy[:, eo * 512:(eo + 1) * 512],
                            on_sb[:, b, ci * 128:(ci + 1) * 128],
                            wo_sb[:, eo * 512:(eo + 1) * 512],
                            start=True, stop=True)
                    ysb = small.tile([128, D], BF, tag="ysb", bufs=4,
                                     name=f"ysb_{b}_{ci}")
                    if _state["drain"] and ci % 2 == 1:
                        nc.scalar.copy(ysb, psy)
                    else:
                        nc.vector.tensor_copy(ysb, psy)
                    nc.sync.dma_start(
                        out=y[b * N + ci * 128: b * N + (ci + 1) * 128, :],
                        in_=ysb)
                return emit

            for b in range(B):
                for tqc in range(NTQC):
                    tq0 = b * N + tqc * TQC  # global tq base
                    exp_t = xbig.tile([128, HPC, NKC, TQC], BF, tag="big")
                    exp0 = small.tile([128, HPC, TQC], BF, tag="exp0",
                                      bufs=2, name=f"exp0_{b}_{tqc}")
                    ots = [ps_o.tile([HD + 1, TQC], F32, tag="ps_o",
                                     name=f"ot_{b}_{tqc}_{h}")
                           for h in range(HPC)]

                    def attn_mms(j):
                        # attn@v for key chunk j (consumes exp of chunk j)
                        for h in range(HPC):
                            vt = v_sb[:, b * NKC + j, h, :]
                            src = exp0[:, h, :] if j == 0 else exp_t[:, h, j, :]
                            for nn in range(TQC // 512):
                                nc.tensor.matmul(
                                    ots[h][:, nn * 512:(nn + 1) * 512], vt,
                                    src[:, nn * 512:(nn + 1) * 512],
                                    start=(j == 0), stop=(j == NKC - 1))

                    # Software-pipelined: the PE's attn@v for chunk tkc-1 is
                    # emitted after exp(tkc) so the PE never waits on the
                    # same-iteration exp; ACT (exp) is the steady-state drum.
                    # One deferred item (prev block's normalization / y-proj
                    # chunk) is woven in per tkc iteration.
                    for tkc in range(NKC):
                        scs = [ps_sc.tile([128, TQC], F32, tag="ps_sc",
                                          name=f"sc_{tkc}_{h}")
                               for h in range(HPC)]
                        # h0/h1 matmuls interleaved: h0 contracts PE rows
                        # 0-63 (tile (0,0)), h1 rows 64-127 (tile (64,0)),
                        # so each adjacent pair runs concurrently.
                        for nn in range(TQC // 512):
                            for h in range(HPC):
                                nc.tensor.matmul(
                                    scs[h][:, nn * 512:(nn + 1) * 512],
                                    k2[h * HD:(h + 1) * HD,
                                       b * N + tkc * 128: b * N + (tkc + 1) * 128],
                                    q2[h * HD:(h + 1) * HD,
                                       tq0 + nn * 512: tq0 + (nn + 1) * 512],
                                    start=True, stop=True)
                        for h in range(HPC):
                            eout = (exp0[:, h, :] if tkc == 0
                                    else exp_t[:, h, tkc, :])
                            nc.scalar.activation(
                                out=eout, in_=scs[h],
                                func=mybir.ActivationFunctionType.Exp,
                                scale=float(HD) ** -0.5)
                        if tkc > 0:
                            attn_mms(tkc - 1)
                        if tkc >= 1 and pending:
                            pending.pop(0)()
                    attn_mms(NKC - 1)

                    # Evacuate the ot PSUM tiles quickly (DVE + gpsimd, one
                    # partition-shifted) so the next block's attn matmuls get
                    # the slots; the reciprocal chain is deferred via
                    # `pending`.
                    # Each head: psum rows 0..63 = out, row 64 = denom.
                    ou2f = small.tile([128, TQC], F32, tag="ou", bufs=2,
                                      name=f"ou_{b}_{tqc}")
                    nc.vector.tensor_copy(ou2f[0:HD, :], ots[0][0:HD, :])
                    nc.scalar.copy(ou2f[HD:2 * HD, :], ots[1][0:HD, :])
                    rss = []
                    for h in range(HPC):
                        rs = small.tile([1, TQC], F32, tag="rs", bufs=4,
                                        name=f"rs_{b}_{tqc}_{h}")
                        nc.vector.tensor_copy(rs, ots[h][HD:HD + 1, :])
                        rss.append(rs)
                    pending.append(norm_item(b, tqc, ou2f, rss))

            # drain remaining deferred work (last block's norms + final ys)
            _state["drain"] = True
            while pending:
                pending.pop(0)()


def _host_inputs(x, Wq, Wk, Wv, Wo, bq, bk, bv, bo):
    """Build the 8 per-core input maps (host-side sharding + layout prep)."""
    bf16 = ml_dtypes.bfloat16
    xTh = np.ascontiguousarray(x.reshape(T, D).T).astype(bf16)

    # rope tables: row e uses freq (e % 64) % 32; positions along columns
    i = (np.arange(E) % HD) % (HD // 2)
    inv_freq = ROPE_BASE ** (-2.0 * i / HD)  # [E]
    ang = np.arange(N)[None, :] * inv_freq[:, None]          # [E, N]
    cosb = np.cos(ang).astype(bf16)
    sinb = np.sin(ang).astype(bf16)

    # rotate-half permutation: rot = P @ q (per 64-block)
    P = np.zeros((E, E), dtype=np.float32)
    for h in range(HPC):
        for j in range(HD // 2):
            P[h * HD + j, h * HD + j + HD // 2] = -1.0
            P[h * HD + j + HD // 2, h * HD + j] = 1.0
    rotT = np.ascontiguousarray(P.T).astype(bf16)
    idT = np.eye(E, dtype=np.float32).astype(bf16)

    in_maps = []
    for c in range(NCORES):
        sl = slice(c * E, (c + 1) * E)
        in_maps.append({
            "xT": xTh,
            "wqT": np.ascontiguousarray(Wq[sl, :].T).astype(bf16),
            "wkT": np.ascontiguousarray(Wk[sl, :].T).astype(bf16),
            "wvT": np.ascontiguousarray(Wv[sl, :].T).astype(bf16),
            "woT": np.ascontiguousarray(Wo[:, sl].T).astype(bf16),
            "bcol": np.stack([bq[sl], bk[sl], bv[sl]], axis=1)
                .astype(np.float32),
            "cosb": cosb,
            "sinb": sinb,
            "rotT": rotT,
            "idT": idT,
        })
    return in_maps


_NC = None


def kernel(x, Wq, Wk, Wv, Wo, bq, bk, bv, bo):
    from concourse.bass_utils import run_bass_kernel_spmd

    global _NC
    if _NC is None:
        _NC = build_nc()
    bo = np.asarray(bo, dtype=np.float32)
    in_maps = _host_inputs(np.asarray(x, dtype=np.float32),
                           np.asarray(Wq, dtype=np.float32),
                           np.asarray(Wk, dtype=np.float32),
                           np.asarray(Wv, dtype=np.float32),
                           np.asarray(Wo, dtype=np.float32),
                           np.asarray(bq, dtype=np.float32),
                           np.asarray(bk, dtype=np.float32),
                           np.asarray(bv, dtype=np.float32),
                           bo)
    res = run_bass_kernel_spmd(_NC, in_maps, core_ids=list(range(NCORES)))
    out = np.zeros((T, D), dtype=np.float32)
    for r in res.results:
        out += np.asarray(r["y"], dtype=np.float32)
    out += bo[None, :]
    return out.reshape(B, N, D)


# revision 11
# speedup vs baseline: 1.2295x; 1.2261x over previous
# Multi-head attention with RoPE, tensor-parallel over heads on 8 NeuronCores.
# v2: 512-wide tq blocks with a composite scores PSUM tile (both heads side
# by side -> ONE exp instruction per key chunk), 6-bank attention drum, and
# the b=1 q/k projections woven through the b=0 attention drum so the PE's
# spare cycles during the ACT-bound drum do useful work.
#
# Layouts (all matmul inputs bf16, fp32 accumulation):
#   xT   [D, T]    : x transposed on host; contraction d on partitions.
#   q2,k2 [E=128,T]: rope'd activations in place (h0 rows 0-63, h1 64-127).
#   v_sb [tk 128, chunk, head, 65] = [v_h | 1] per head (ones -> denom row).
#   scores^T       : composite PSUM tile [128, 1024] per tkc: cols 0-511 =
#                    h0 (tile (0,0), contracts partitions 0-63), cols
#                    512-1023 = h1 (tile (64,0)) -> the two matmuls run
#                    CONCURRENTLY on disjoint PE row groups; ONE 1024-wide
#                    exp instruction serves both heads.
#   attn@v         : ot_h [65, 512] PSUM accumulators (1 bank each).
#   norm           : denom row DVE-copied out, reciprocal + gpsimd broadcast,
#                    one [128,512] DVE mul into on_sb.
#   out-proj       : y[t, :] = on^T @ woT per 128-token chunk, psum halves
#                    from the shared weave pool, bf16 staged, DMA'd out.

import numpy as np
import ml_dtypes

import concourse.bass as bass
import concourse.mybir as mybir
import concourse.tile as tile
from concourse import bacc

B, N, D, H = 2, 2048, 1024, 16
HD = 64
T = B * N                 # 4096 tokens
NCORES = 8
HPC = H // NCORES         # 2 heads per core
E = HPC * HD              # 128 per-core projection columns
KD = D // 128             # 8 contraction tiles for d
ROPE_BASE = 10000.0

BF = mybir.dt.bfloat16
F32 = mybir.dt.float32

TQB = 512                 # tq block (psum width per head)
NB = N // TQB             # 4 blocks per batch
NKC = N // 128            # 16 key chunks per batch
EW = 2 * TQB              # composite exp width (both heads)


def build_nc():
    nc = bacc.Bacc(trn_type="TRN2", target_bir_lowering=False, debug=False)

    xT = nc.dram_tensor("xT", [D, T], BF, kind="ExternalInput").ap()
    wqT = nc.dram_tensor("wqT", [D, E], BF, kind="ExternalInput").ap()
    wkT = nc.dram_tensor("wkT", [D, E], BF, kind="ExternalInput").ap()
    wvT = nc.dram_tensor("wvT", [D, E], BF, kind="ExternalInput").ap()
    woT = nc.dram_tensor("woT", [E, D], BF, kind="ExternalInput").ap()
    bcol = nc.dram_tensor("bcol", [E, 3], F32, kind="ExternalInput").ap()
    cosb = nc.dram_tensor("cosb", [E, N], BF, kind="ExternalInput").ap()
    sinb = nc.dram_tensor("sinb", [E, N], BF, kind="ExternalInput").ap()
    rotT = nc.dram_tensor("rotT", [E, E], BF, kind="ExternalInput").ap()
    idT = nc.dram_tensor("idT", [E, E], BF, kind="ExternalInput").ap()
    y = nc.dram_tensor("y", [T, D], BF, kind="ExternalOutput").ap()

    with tile.TileContext(nc) as tc:
        _build(tc, nc, xT, wqT, wkT, wvT, woT, bcol, cosb, sinb,
               rotT, idT, y)
    nc.compile()
    return nc


def _build(tc, nc, xT, wqT, wkT, wvT, woT, bcol, cosb, sinb,
           rotT, idT, y):
    with (
        tc.tile_pool(name="consts", bufs=1) as consts,
        tc.tile_pool(name="xbig", bufs=1) as xbig,
        tc.tile_pool(name="ebig", bufs=1) as ebig,
        tc.tile_pool(name="acts", bufs=1) as acts,
        tc.tile_pool(name="small", bufs=3) as small,
    ):
        # ---- constants / weights ----
        wq_sb = consts.tile([128, KD, E], BF, tag="wq")
        wk_sb = consts.tile([128, KD, E], BF, tag="wk")
        wv_sb = consts.tile([128, KD, E], BF, tag="wv")
        nc.sync.dma_start(out=wq_sb, in_=wqT.rearrange("(k p) e -> p k e", p=128))
        x_sb = xbig.tile([128, KD, T], BF, tag="big")
        xTr = xT.rearrange("(k p) t -> p k t", p=128)
        nc.sync.dma_start(out=x_sb[:, :, 0:512], in_=xTr[:, :, 0:512])
        nc.sync.dma_start(out=wk_sb, in_=wkT.rearrange("(k p) e -> p k e", p=128))
        nc.sync.dma_start(out=wv_sb, in_=wvT.rearrange("(k p) e -> p k e", p=128))
        nc.sync.dma_start(out=x_sb[:, :, 512:1024], in_=xTr[:, :, 512:1024])
        wo_sb = consts.tile([E, D], BF, tag="wo")
        nc.sync.dma_start(out=wo_sb, in_=woT)
        bcol_sb = consts.tile([E, 3], F32, tag="bcol")
        nc.sync.dma_start(out=bcol_sb, in_=bcol)
        cos_sb = consts.tile([E, N], BF, tag="cos")
        sin_sb = consts.tile([E, N], BF, tag="sin")
        nc.sync.dma_start(out=cos_sb, in_=cosb)
        nc.sync.dma_start(out=sin_sb, in_=sinb)
        rot_sb = consts.tile([E, E], BF, tag="rot")
        nc.sync.dma_start(out=rot_sb, in_=rotT)
        id_sb = consts.tile([E, E], BF, tag="idT")
        nc.sync.dma_start(out=id_sb, in_=idT)

        for ci in range(2, T // 512):
            nc.sync.dma_start(out=x_sb[:, :, ci * 512:(ci + 1) * 512],
                              in_=xTr[:, :, ci * 512:(ci + 1) * 512])

        # ---- persistent activations ----
        q2 = acts.tile([E, T], BF, tag="q2")
        k2 = acts.tile([E, T], BF, tag="k2")
        v_sb = acts.tile([128, T // 128, HPC, HD + 1], BF, tag="v_sb")
        on_sb = acts.tile([E, B, N], BF, tag="on_sb")

        nc.vector.memset(v_sb[:, :, :, HD:HD + 1], 1.0)

        # ---------- projection emitters (used in phase 1a and the weave) ----
        def proj_qk_slice(ps_pool, psr_pool, ci, dst, w, bc, granules=None):
            # q or k projection for 512-token slice ci, rope'd in place.
            # When `granules` is a list, work is appended as deferred items.
            # PSUM tiles are allocated inside the granule bodies so pool
            # slot rotation matches emission order.
            sl = slice(ci * 512, (ci + 1) * 512)
            npos = (ci * 512) % N
            tsl = slice(npos, npos + 512)
            cell = {}

            def part_a():
                ps = ps_pool.tile([128, 512], F32, tag="ps_qk",
                                  name=f"ps_{bc}_{ci}")
                cell["ps"] = ps
                for k in range(KD // 2):
                    nc.tensor.matmul(ps, w[:, k, :], x_sb[:, k, sl],
                                     start=(k == 0), stop=False)

            def part_b():
                ps = cell["ps"]
                for k in range(KD // 2, KD):
                    nc.tensor.matmul(ps, w[:, k, :], x_sb[:, k, sl],
                                     start=False, stop=(k == KD - 1))
                nc.vector.tensor_scalar_add(dst[:, sl], ps, bcol_sb[:, bc:bc + 1])

            def part_c():
                psr = psr_pool.tile([128, 512], F32, tag="ps_qk",
                                    name=f"psr_{bc}_{ci}")
                nc.tensor.matmul(psr, rot_sb, dst[:, sl], start=True, stop=True)
                t1 = small.tile([128, 512], BF, tag="rope_t1")
                nc.vector.tensor_mul(t1, dst[:, sl], cos_sb[:, tsl])
                t2 = small.tile([128, 512], BF, tag="rope_t2")
                nc.vector.tensor_mul(t2, psr, sin_sb[:, tsl])
                nc.vector.tensor_add(dst[:, sl], t1, t2)

            if granules is None:
                part_a(); part_b(); part_c()
            else:
                granules.extend([part_a, part_b, part_c])

        def proj_v_slice(ps_pool, pst_pool, ci):
            sl = slice(ci * 512, (ci + 1) * 512)
            psv = ps_pool.tile([128, 512], F32, tag="ps_qk", name=f"psv_{ci}")
            for k in range(KD):
                nc.tensor.matmul(psv, wv_sb[:, k, :], x_sb[:, k, sl],
                                 start=(k == 0), stop=(k == KD - 1))
            vts = small.tile([128, 512], BF, tag="vts")
            nc.vector.tensor_scalar_add(vts, psv, bcol_sb[:, 2:3])
            for s in range(4):
                cv = ci * 4 + s
                pst = pst_pool.tile([128, 128], BF, tag="ps_t",
                                    name=f"pst_{ci}_{s}")
                nc.tensor.transpose(pst, vts[:, s * 128:(s + 1) * 128], id_sb)
                nc.vector.tensor_copy(v_sb[:, cv, :, 0:HD], pst)

        # ================= phase 1a =================
        # b=0 projections (slices 0-3) fully, plus v for b=1 (slices 4-7,
        # transposes need their own psum which the drum can't spare).
        with (
            tc.tile_pool(name="ps_qk", bufs=2, space="PSUM") as ps_qk,
            tc.tile_pool(name="ps_r", bufs=2, space="PSUM") as ps_r,
            tc.tile_pool(name="ps_t", bufs=2, space="PSUM") as ps_t,
        ):
            for ci in range(4):
                proj_qk_slice(ps_qk, ps_r, ci, k2, wk_sb, 1)
                proj_v_slice(ps_qk, ps_t, ci)
                proj_qk_slice(ps_qk, ps_r, ci, q2, wq_sb, 0)
            for ci in range(4, 8):
                proj_v_slice(ps_qk, ps_t, ci)

        # ========= phase 2: attention drum + woven work =========
        with (
            tc.tile_pool(name="ps_sc", bufs=2, space="PSUM") as ps_sc,
            tc.tile_pool(name="ps_o", bufs=2, space="PSUM") as ps_o,
            tc.tile_pool(name="ps_w", bufs=2, space="PSUM") as ps_w,
        ):
            critical = []   # b=1 q/k projection granules (must finish in b0)
            pending = []    # norm / out-proj items (may spill)
            _state = {"drain": False}

            # enqueue the b=1 q/k projection granules
            for ci in range(4, 8):
                proj_qk_slice(ps_w, ps_w, ci, k2, wk_sb, 1, granules=critical)
            for ci in range(4, 8):
                proj_qk_slice(ps_w, ps_w, ci, q2, wq_sb, 0, granules=critical)

            def norm_item(b, qb, ou2f, rss):
                def emit():
                    for h in range(HPC):
                        rc = small.tile([1, TQB], F32, tag="recip", bufs=2,
                                        name=f"rc_{b}_{qb}_{h}")
                        nc.vector.reciprocal_approx_fast(out=rc, in_=rss[h])
                        rbs = small.tile([128, TQB], F32, tag="recipb",
                                         bufs=2, name=f"rbs_{b}_{qb}_{h}")
                        nc.gpsimd.partition_broadcast(
                            out_ap=rbs, in_ap=rc, channels=128)
                        hsl = slice(h * HD, (h + 1) * HD)
                        nc.vector.tensor_mul(
                            on_sb[hsl, b, qb * TQB:(qb + 1) * TQB],
                            ou2f[hsl, :], rbs[hsl, :])
                    for ci in range(qb * 4, qb * 4 + 4):
                        pending.append(y_item(b, ci))
                return emit

            def y_item(b, ci):
                def emit():
                    ysb = small.tile([128, D], BF, tag="ysb", bufs=4,
                                     name=f"ysb_{b}_{ci}")
                    for eo in range(D // 512):
                        psy = ps_w.tile([128, 512], F32, tag="ps_qk",
                                        name=f"psy_{b}_{ci}_{eo}")
                        nc.tensor.matmul(
                            psy,
                            on_sb[:, b, ci * 128:(ci + 1) * 128],
                            wo_sb[:, eo * 512:(eo + 1) * 512],
                            start=True, stop=True)
                        if _state["drain"] and (ci + eo) % 2 == 1:
                            nc.scalar.copy(ysb[:, eo * 512:(eo + 1) * 512], psy)
                        else:
                            nc.vector.tensor_copy(
                                ysb[:, eo * 512:(eo + 1) * 512], psy)
                    nc.sync.dma_start(
                        out=y[b * N + ci * 128: b * N + (ci + 1) * 128, :],
                        in_=ysb)
                return emit

            def pop_item():
                if critical:
                    critical.pop(0)()
                elif pending:
                    pending.pop(0)()

            for b in range(B):
                for qb in range(NB):
                    tq0 = b * N + qb * TQB
                    exp_t = ebig.tile([128, NKC, EW], BF, tag="exp",
                                      name=f"exp_{b}_{qb}")
                    exp0 = small.tile([128, EW], BF, tag="exp0", bufs=2,
                                      name=f"exp0_{b}_{qb}")
                    ots = [ps_o.tile([HD + 1, TQB], F32, tag="ps_o",
                                     name=f"ot_{b}_{qb}_{h}")
                           for h in range(HPC)]

                    def attn_mms(j):
                        for h in range(HPC):
                            src = (exp0 if j == 0 else exp_t[:, j, :])
                            nc.tensor.matmul(
                                ots[h], v_sb[:, b * NKC + j, h, :],
                                src[:, h * TQB:(h + 1) * TQB],
                                start=(j == 0), stop=(j == NKC - 1))

                    for tkc in range(NKC):
                        sc = ps_sc.tile([128, EW], F32, tag="ps_sc",
                                        name=f"sc_{b}_{qb}_{tkc}")
                        # h0 -> cols 0-511 (PE tile (0,0)), h1 -> cols
                        # 512-1023 (PE tile (64,0)): concurrent matmuls.
                        for h in range(HPC):
                            nc.tensor.matmul(
                                sc[:, h * TQB:(h + 1) * TQB],
                                k2[h * HD:(h + 1) * HD,
                                   b * N + tkc * 128: b * N + (tkc + 1) * 128],
                                q2[h * HD:(h + 1) * HD, tq0: tq0 + TQB],
                                start=True, stop=True)
                        nc.scalar.activation(
                            out=(exp0 if tkc == 0 else exp_t[:, tkc, :]),
                            in_=sc,
                            func=mybir.ActivationFunctionType.Exp,
                            scale=float(HD) ** -0.5)
                        if tkc > 0:
                            attn_mms(tkc - 1)
                        if tkc >= 1:
                            pop_item()
                    attn_mms(NKC - 1)

                    # fast PSUM evacuation (DVE + gpsimd); recip chain deferred
                    ou2f = small.tile([128, TQB], F32, tag="ou", bufs=2,
                                      name=f"ou_{b}_{qb}")
                    nc.vector.tensor_copy(ou2f[0:HD, :], ots[0][0:HD, :])
                    nc.vector.tensor_copy(ou2f[HD:2 * HD, :], ots[1][0:HD, :])
                    rss = []
                    for h in range(HPC):
                        rs = small.tile([1, TQB], F32, tag="rs", bufs=4,
                                        name=f"rs_{b}_{qb}_{h}")
                        nc.vector.tensor_copy(rs, ots[h][HD:HD + 1, :])
                        rss.append(rs)
                    pending.append(norm_item(b, qb, ou2f, rss))

            _state["drain"] = True
            while critical:
                critical.pop(0)()
            while pending:
                pending.pop(0)()


def _host_inputs(x, Wq, Wk, Wv, Wo, bq, bk, bv, bo):
    """Build the 8 per-core input maps (host-side sharding + layout prep)."""
    bf16 = ml_dtypes.bfloat16
    xTh = np.ascontiguousarray(x.reshape(T, D).T).astype(bf16)

    i = (np.arange(E) % HD) % (HD // 2)
    inv_freq = ROPE_BASE ** (-2.0 * i / HD)  # [E]
    ang = np.arange(N)[None, :] * inv_freq[:, None]          # [E, N]
    cosb = np.cos(ang).astype(bf16)
    sinb = np.sin(ang).astype(bf16)

    P = np.zeros((E, E), dtype=np.float32)
    for h in range(HPC):
        for j in range(HD // 2):
            P[h * HD + j, h * HD + j + HD // 2] = -1.0
            P[h * HD + j + HD // 2, h * HD + j] = 1.0
    rotT = np.ascontiguousarray(P.T).astype(bf16)
    idT = np.eye(E, dtype=np.float32).astype(bf16)

    in_maps = []
    for c in range(NCORES):
        sl = slice(c * E, (c + 1) * E)
        in_maps.append({
            "xT": xTh,
            "wqT": np.ascontiguousarray(Wq[sl, :].T).astype(bf16),
            "wkT": np.ascontiguousarray(Wk[sl, :].T).astype(bf16),
            "wvT": np.ascontiguousarray(Wv[sl, :].T).astype(bf16),
            "woT": np.ascontiguousarray(Wo[:, sl].T).astype(bf16),
            "bcol": np.stack([bq[sl], bk[sl], bv[sl]], axis=1)
                .astype(np.float32),
            "cosb": cosb,
            "sinb": sinb,
            "rotT": rotT,
            "idT": idT,
        })
    return in_maps


_NC = None


def kernel(x, Wq, Wk, Wv, Wo, bq, bk, bv, bo):
    from concourse.bass_utils import run_bass_kernel_spmd

    global _NC
    if _NC is None:
        _NC = build_nc()
    bo = np.asarray(bo, dtype=np.float32)
    in_maps = _host_inputs(np.asarray(x, dtype=np.float32),
                           np.asarray(Wq, dtype=np.float32),
                           np.asarray(Wk, dtype=np.float32),
                           np.asarray(Wv, dtype=np.float32),
                           np.asarray(Wo, dtype=np.float32),
                           np.asarray(bq, dtype=np.float32),
                           np.asarray(bk, dtype=np.float32),
                           np.asarray(bv, dtype=np.float32),
                           bo)
    res = run_bass_kernel_spmd(_NC, in_maps, core_ids=list(range(NCORES)))
    out = np.zeros((T, D), dtype=np.float32)
    for r in res.results:
        out += np.asarray(r["y"], dtype=np.float32)
    out += bo[None, :]
    return out.reshape(B, N, D)


# revision 12
# speedup vs baseline: 1.2297x; 1.0002x over previous
# Multi-head attention with RoPE, tensor-parallel over heads on 8 NeuronCores.
# v2: 512-wide tq blocks with a composite scores PSUM tile (both heads side
# by side -> ONE exp instruction per key chunk), 6-bank attention drum, and
# the b=1 q/k projections woven through the b=0 attention drum so the PE's
# spare cycles during the ACT-bound drum do useful work.
#
# Layouts (all matmul inputs bf16, fp32 accumulation):
#   xT   [D, T]    : x transposed on host; contraction d on partitions.
#   q2,k2 [E=128,T]: rope'd activations in place (h0 rows 0-63, h1 64-127).
#   v_sb [tk 128, chunk, head, 65] = [v_h | 1] per head (ones -> denom row).
#   scores^T       : composite PSUM tile [128, 1024] per tkc: cols 0-511 =
#                    h0 (tile (0,0), contracts partitions 0-63), cols
#                    512-1023 = h1 (tile (64,0)) -> the two matmuls run
#                    CONCURRENTLY on disjoint PE row groups; ONE 1024-wide
#                    exp instruction serves both heads.
#   attn@v         : ot_h [65, 512] PSUM accumulators (1 bank each).
#   norm           : denom row DVE-copied out, reciprocal + gpsimd broadcast,
#                    one [128,512] DVE mul into on_sb.
#   out-proj       : y[t, :] = on^T @ woT per 128-token chunk, psum halves
#                    from the shared weave pool, bf16 staged, DMA'd out.

import numpy as np
import ml_dtypes

import concourse.bass as bass
import concourse.mybir as mybir
import concourse.tile as tile
from concourse import bacc

B, N, D, H = 2, 2048, 1024, 16
HD = 64
T = B * N                 # 4096 tokens
NCORES = 8
HPC = H // NCORES         # 2 heads per core
E = HPC * HD              # 128 per-core projection columns
KD = D // 128             # 8 contraction tiles for d
ROPE_BASE = 10000.0

BF = mybir.dt.bfloat16
F32 = mybir.dt.float32

TQB = 512                 # tq block (psum width per head)
NB = N // TQB             # 4 blocks per batch
NKC = N // 128            # 16 key chunks per batch
EW = 2 * TQB              # composite exp width (both heads)


def build_nc():
    nc = bacc.Bacc(trn_type="TRN2", target_bir_lowering=False, debug=False)

    xT = nc.dram_tensor("xT", [D, T], BF, kind="ExternalInput").ap()
    wqT = nc.dram_tensor("wqT", [D, E], BF, kind="ExternalInput").ap()
    wkT = nc.dram_tensor("wkT", [D, E], BF, kind="ExternalInput").ap()
    wvT = nc.dram_tensor("wvT", [D, E], BF, kind="ExternalInput").ap()
    woT = nc.dram_tensor("woT", [E, D], BF, kind="ExternalInput").ap()
    bcol = nc.dram_tensor("bcol", [E, 3], F32, kind="ExternalInput").ap()
    cosb = nc.dram_tensor("cosb", [E, N], BF, kind="ExternalInput").ap()
    sinb = nc.dram_tensor("sinb", [E, N], BF, kind="ExternalInput").ap()
    rotT = nc.dram_tensor("rotT", [E, E], BF, kind="ExternalInput").ap()
    idT = nc.dram_tensor("idT", [E, E], BF, kind="ExternalInput").ap()
    y = nc.dram_tensor("y", [T, D], BF, kind="ExternalOutput").ap()

    with tile.TileContext(nc) as tc:
        _build(tc, nc, xT, wqT, wkT, wvT, woT, bcol, cosb, sinb,
               rotT, idT, y)
    nc.compile()
    return nc


def _build(tc, nc, xT, wqT, wkT, wvT, woT, bcol, cosb, sinb,
           rotT, idT, y):
    with (
        tc.tile_pool(name="consts", bufs=1) as consts,
        tc.tile_pool(name="xbig", bufs=1) as xbig,
        tc.tile_pool(name="ebig", bufs=1) as ebig,
        tc.tile_pool(name="acts", bufs=1) as acts,
        tc.tile_pool(name="small", bufs=3) as small,
    ):
        # ---- constants / weights ----
        wq_sb = consts.tile([128, KD, E], BF, tag="wq")
        wk_sb = consts.tile([128, KD, E], BF, tag="wk")
        wv_sb = consts.tile([128, KD, E], BF, tag="wv")
        nc.sync.dma_start(out=wq_sb, in_=wqT.rearrange("(k p) e -> p k e", p=128))
        x_sb = xbig.tile([128, KD, T], BF, tag="big")
        xTr = xT.rearrange("(k p) t -> p k t", p=128)
        nc.sync.dma_start(out=x_sb[:, :, 0:512], in_=xTr[:, :, 0:512])
        nc.sync.dma_start(out=wk_sb, in_=wkT.rearrange("(k p) e -> p k e", p=128))
        nc.sync.dma_start(out=wv_sb, in_=wvT.rearrange("(k p) e -> p k e", p=128))
        nc.sync.dma_start(out=x_sb[:, :, 512:1024], in_=xTr[:, :, 512:1024])
        wo_sb = consts.tile([E, D], BF, tag="wo")
        nc.sync.dma_start(out=wo_sb, in_=woT)
        bcol_sb = consts.tile([E, 3], F32, tag="bcol")
        nc.sync.dma_start(out=bcol_sb, in_=bcol)
        cos_sb = consts.tile([E, N], BF, tag="cos")
        sin_sb = consts.tile([E, N], BF, tag="sin")
        nc.sync.dma_start(out=cos_sb, in_=cosb)
        nc.sync.dma_start(out=sin_sb, in_=sinb)
        rot_sb = consts.tile([E, E], BF, tag="rot")
        nc.sync.dma_start(out=rot_sb, in_=rotT)
        id_sb = consts.tile([E, E], BF, tag="idT")
        nc.sync.dma_start(out=id_sb, in_=idT)

        for ci in range(2, T // 512):
            nc.sync.dma_start(out=x_sb[:, :, ci * 512:(ci + 1) * 512],
                              in_=xTr[:, :, ci * 512:(ci + 1) * 512])

        # ---- persistent activations ----
        q2 = acts.tile([E, T], BF, tag="q2")
        k2 = acts.tile([E, T], BF, tag="k2")
        v_sb = acts.tile([128, T // 128, HPC, HD + 1], BF, tag="v_sb")
        on_sb = acts.tile([E, B, N], BF, tag="on_sb")

        nc.vector.memset(v_sb[:, :, :, HD:HD + 1], 1.0)

        # ---------- projection emitters (used in phase 1a and the weave) ----
        def proj_qk_slice(ps_pool, psr_pool, ci, dst, w, bc, granules=None):
            # q or k projection for 512-token slice ci, rope'd in place.
            # When `granules` is a list, work is appended as deferred items.
            # PSUM tiles are allocated inside the granule bodies so pool
            # slot rotation matches emission order.
            sl = slice(ci * 512, (ci + 1) * 512)
            npos = (ci * 512) % N
            tsl = slice(npos, npos + 512)
            cell = {}

            def part_a():
                ps = ps_pool.tile([128, 512], F32, tag="ps_qk",
                                  name=f"ps_{bc}_{ci}")
                cell["ps"] = ps
                for k in range(KD // 2):
                    nc.tensor.matmul(ps, w[:, k, :], x_sb[:, k, sl],
                                     start=(k == 0), stop=False)

            def part_b():
                ps = cell["ps"]
                for k in range(KD // 2, KD):
                    nc.tensor.matmul(ps, w[:, k, :], x_sb[:, k, sl],
                                     start=False, stop=(k == KD - 1))
                nc.vector.tensor_scalar_add(dst[:, sl], ps, bcol_sb[:, bc:bc + 1])

            def part_c():
                psr = psr_pool.tile([128, 512], F32, tag="ps_qk",
                                    name=f"psr_{bc}_{ci}")
                nc.tensor.matmul(psr, rot_sb, dst[:, sl], start=True, stop=True)
                t1 = small.tile([128, 512], BF, tag="rope_t1")
                nc.vector.tensor_mul(t1, dst[:, sl], cos_sb[:, tsl])
                t2 = small.tile([128, 512], BF, tag="rope_t2")
                nc.vector.tensor_mul(t2, psr, sin_sb[:, tsl])
                nc.vector.tensor_add(dst[:, sl], t1, t2)

            if granules is None:
                part_a(); part_b(); part_c()
            else:
                granules.extend([part_a, part_b, part_c])

        def proj_v_slice(ps_pool, pst_pool, ci):
            sl = slice(ci * 512, (ci + 1) * 512)
            psv = ps_pool.tile([128, 512], F32, tag="ps_qk", name=f"psv_{ci}")
            for k in range(KD):
                nc.tensor.matmul(psv, wv_sb[:, k, :], x_sb[:, k, sl],
                                 start=(k == 0), stop=(k == KD - 1))
            vts = small.tile([128, 512], BF, tag="vts")
            nc.vector.tensor_scalar_add(vts, psv, bcol_sb[:, 2:3])
            for s in range(4):
                cv = ci * 4 + s
                pst = pst_pool.tile([128, 128], BF, tag="ps_t",
                                    name=f"pst_{ci}_{s}")
                nc.tensor.transpose(pst, vts[:, s * 128:(s + 1) * 128], id_sb)
                nc.vector.tensor_copy(v_sb[:, cv, :, 0:HD], pst)

        # ================= phase 1a =================
        # b=0 projections (slices 0-3) fully, plus v for b=1 (slices 4-7,
        # transposes need their own psum which the drum can't spare).
        with (
            tc.tile_pool(name="ps_qk", bufs=2, space="PSUM") as ps_qk,
            tc.tile_pool(name="ps_r", bufs=2, space="PSUM") as ps_r,
            tc.tile_pool(name="ps_t", bufs=2, space="PSUM") as ps_t,
        ):
            for ci in range(4):
                proj_qk_slice(ps_qk, ps_r, ci, k2, wk_sb, 1)
                proj_v_slice(ps_qk, ps_t, ci)
                proj_qk_slice(ps_qk, ps_r, ci, q2, wq_sb, 0)
            for ci in range(4, 8):
                proj_v_slice(ps_qk, ps_t, ci)

        # ========= phase 2: attention drum + woven work =========
        with (
            tc.tile_pool(name="ps_sc", bufs=2, space="PSUM") as ps_sc,
            tc.tile_pool(name="ps_o", bufs=2, space="PSUM") as ps_o,
            tc.tile_pool(name="ps_w", bufs=2, space="PSUM") as ps_w,
        ):
            critical = []   # b=1 q/k projection granules (must finish in b0)
            pending = []    # norm / out-proj items (may spill)
            _state = {"drain": False}

            # enqueue the b=1 q/k projection granules
            for ci in range(4, 8):
                proj_qk_slice(ps_w, ps_w, ci, k2, wk_sb, 1, granules=critical)
            for ci in range(4, 8):
                proj_qk_slice(ps_w, ps_w, ci, q2, wq_sb, 0, granules=critical)

            def norm_item(b, qb, ou2f, rss):
                def emit():
                    for h in range(HPC):
                        rc = small.tile([1, TQB], F32, tag="recip", bufs=2,
                                        name=f"rc_{b}_{qb}_{h}")
                        nc.vector.reciprocal_approx_fast(out=rc, in_=rss[h])
                        rbs = small.tile([128, TQB], F32, tag="recipb",
                                         bufs=2, name=f"rbs_{b}_{qb}_{h}")
                        nc.gpsimd.partition_broadcast(
                            out_ap=rbs, in_ap=rc, channels=128)
                        hsl = slice(h * HD, (h + 1) * HD)
                        nc.vector.tensor_mul(
                            on_sb[hsl, b, qb * TQB:(qb + 1) * TQB],
                            ou2f[hsl, :], rbs[hsl, :])
                    for ci in range(qb * 4, qb * 4 + 4):
                        pending.append(y_item(b, ci))
                return emit

            def y_item(b, ci):
                def emit():
                    ysb = small.tile([128, D], BF, tag="ysb", bufs=4,
                                     name=f"ysb_{b}_{ci}")
                    for eo in range(D // 512):
                        psy = ps_w.tile([128, 512], F32, tag="ps_qk",
                                        name=f"psy_{b}_{ci}_{eo}")
                        nc.tensor.matmul(
                            psy,
                            on_sb[:, b, ci * 128:(ci + 1) * 128],
                            wo_sb[:, eo * 512:(eo + 1) * 512],
                            start=True, stop=True)
                        if _state["drain"] and (ci + eo) % 2 == 1:
                            nc.scalar.copy(ysb[:, eo * 512:(eo + 1) * 512], psy)
                        else:
                            nc.vector.tensor_copy(
                                ysb[:, eo * 512:(eo + 1) * 512], psy)
                    nc.sync.dma_start(
                        out=y[b * N + ci * 128: b * N + (ci + 1) * 128, :],
                        in_=ysb)
                return emit

            def pop_item():
                if critical:
                    critical.pop(0)()
                elif pending:
                    pending.pop(0)()

            # ---- flat global software pipeline over (b, qb, tkc) ----
            # Per iteration i: emit scores(i+1) [one period AHEAD of the
            # exp that consumes it, so ACT never waits on the PE FIFO],
            # exp(i), attn(i-1), block-end evacuation, then one deferred
            # item. This keeps the exp drum gap-free across block
            # boundaries too.
            triples = [(b, qb, tkc)
                       for b in range(B) for qb in range(NB)
                       for tkc in range(NKC)]
            NTR = len(triples)
            blk_state = {}
            sc_of = {}

            def emit_scores(i):
                b, qb, tkc = triples[i]
                tq0 = b * N + qb * TQB
                sc = ps_sc.tile([128, EW], F32, tag="ps_sc",
                                name=f"sc_{b}_{qb}_{tkc}")
                sc_of[i] = sc
                # h0 -> cols 0-511 (PE tile (0,0)), h1 -> cols 512-1023
                # (PE tile (64,0)): concurrent matmuls.
                for h in range(HPC):
                    nc.tensor.matmul(
                        sc[:, h * TQB:(h + 1) * TQB],
                        k2[h * HD:(h + 1) * HD,
                           b * N + tkc * 128: b * N + (tkc + 1) * 128],
                        q2[h * HD:(h + 1) * HD, tq0: tq0 + TQB],
                        start=True, stop=True)

            def emit_exp(i):
                b, qb, tkc = triples[i]
                st = blk_state.setdefault((b, qb), {})
                if tkc == 0:
                    st["exp0"] = small.tile([128, EW], BF, tag="exp0",
                                            bufs=2, name=f"exp0_{b}_{qb}")
                    eout = st["exp0"]
                else:
                    if "exp_t" not in st:
                        st["exp_t"] = ebig.tile([128, NKC, EW], BF,
                                                tag="exp",
                                                name=f"exp_{b}_{qb}")
                    eout = st["exp_t"][:, tkc, :]
                nc.scalar.activation(
                    out=eout, in_=sc_of.pop(i),
                    func=mybir.ActivationFunctionType.Exp,
                    scale=float(HD) ** -0.5)

            def emit_attn(i):
                b, qb, tkc = triples[i]
                st = blk_state[(b, qb)]
                if tkc == 0:
                    st["ots"] = [ps_o.tile([HD + 1, TQB], F32, tag="ps_o",
                                           name=f"ot_{b}_{qb}_{h}")
                                 for h in range(HPC)]
                src = st["exp0"] if tkc == 0 else st["exp_t"][:, tkc, :]
                for h in range(HPC):
                    nc.tensor.matmul(
                        st["ots"][h], v_sb[:, b * NKC + tkc, h, :],
                        src[:, h * TQB:(h + 1) * TQB],
                        start=(tkc == 0), stop=(tkc == NKC - 1))
                if tkc == NKC - 1:
                    emit_evac(b, qb, st)

            def emit_evac(b, qb, st):
                # fast PSUM evacuation (DVE); recip chain deferred
                ots = st["ots"]
                ou2f = small.tile([128, TQB], F32, tag="ou", bufs=2,
                                  name=f"ou_{b}_{qb}")
                nc.vector.tensor_copy(ou2f[0:HD, :], ots[0][0:HD, :])
                nc.vector.tensor_copy(ou2f[HD:2 * HD, :], ots[1][0:HD, :])
                rss = []
                for h in range(HPC):
                    rs = small.tile([1, TQB], F32, tag="rs", bufs=4,
                                    name=f"rs_{b}_{qb}_{h}")
                    nc.vector.tensor_copy(rs, ots[h][HD:HD + 1, :])
                    rss.append(rs)
                pending.append(norm_item(b, qb, ou2f, rss))

            for i in range(NTR):
                if i == 0:
                    emit_scores(0)
                if i + 1 < NTR:
                    emit_scores(i + 1)
                emit_exp(i)
                if i > 0:
                    emit_attn(i - 1)
                if i >= 1:
                    pop_item()
            emit_attn(NTR - 1)

            _state["drain"] = True
            while critical:
                critical.pop(0)()
            while pending:
                pending.pop(0)()


def _host_inputs(x, Wq, Wk, Wv, Wo, bq, bk, bv, bo):
    """Build the 8 per-core input maps (host-side sharding + layout prep)."""
    bf16 = ml_dtypes.bfloat16
    xTh = np.ascontiguousarray(x.reshape(T, D).T).astype(bf16)

    i = (np.arange(E) % HD) % (HD // 2)
    inv_freq = ROPE_BASE ** (-2.0 * i / HD)  # [E]
    ang = np.arange(N)[None, :] * inv_freq[:, None]          # [E, N]
    cosb = np.cos(ang).astype(bf16)
    sinb = np.sin(ang).astype(bf16)

    P = np.zeros((E, E), dtype=np.float32)
    for h in range(HPC):
        for j in range(HD // 2):
            P[h * HD + j, h * HD + j + HD // 2] = -1.0
            P[h * HD + j + HD // 2, h * HD + j] = 1.0
    rotT = np.ascontiguousarray(P.T).astype(bf16)
    idT = np.eye(E, dtype=np.float32).astype(bf16)

    in_maps = []
    for c in range(NCORES):
        sl = slice(c * E, (c + 1) * E)
        in_maps.append({
            "xT": xTh,
            "wqT": np.ascontiguousarray(Wq[sl, :].T).astype(bf16),
            "wkT": np.ascontiguousarray(Wk[sl, :].T).astype(bf16),
            "wvT": np.ascontiguousarray(Wv[sl, :].T).astype(bf16),
            "woT": np.ascontiguousarray(Wo[:, sl].T).astype(bf16),
            "bcol": np.stack([bq[sl], bk[sl], bv[sl]], axis=1)
                .astype(np.float32),
            "cosb": cosb,
            "sinb": sinb,
            "rotT": rotT,
            "idT": idT,
        })
    return in_maps


_NC = None


def kernel(x, Wq, Wk, Wv, Wo, bq, bk, bv, bo):
    from concourse.bass_utils import run_bass_kernel_spmd

    global _NC
    if _NC is None:
        _NC = build_nc()
    bo = np.asarray(bo, dtype=np.float32)
    in_maps = _host_inputs(np.asarray(x, dtype=np.float32),
                           np.asarray(Wq, dtype=np.float32),
                           np.asarray(Wk, dtype=np.float32),
                           np.asarray(Wv, dtype=np.float32),
                           np.asarray(Wo, dtype=np.float32),
                           np.asarray(bq, dtype=np.float32),
                           np.asarray(bk, dtype=np.float32),
                           np.asarray(bv, dtype=np.float32),
                           bo)
    res = run_bass_kernel_spmd(_NC, in_maps, core_ids=list(range(NCORES)))
    out = np.zeros((T, D), dtype=np.float32)
    for r in res.results:
        out += np.asarray(r["y"], dtype=np.float32)
    out += bo[None, :]
    return out.reshape(B, N, D)


# revision 18
# speedup vs baseline: 1.3703x; 1.1144x over previous
# Multi-head attention with RoPE, tensor-parallel over heads on 8 NeuronCores.
# v2: 512-wide tq blocks with a composite scores PSUM tile (both heads side
# by side -> ONE exp instruction per key chunk), 6-bank attention drum, and
# the b=1 q/k projections woven through the b=0 attention drum so the PE's
# spare cycles during the ACT-bound drum do useful work.
#
# Layouts (all matmul inputs bf16, fp32 accumulation):
#   xT   [D, T]    : x transposed on host; contraction d on partitions.
#   q2,k2 [E=128,T]: rope'd activations in place (h0 rows 0-63, h1 64-127).
#   v_sb [tk 128, chunk, head, 65] = [v_h | 1] per head (ones -> denom row).
#   scores^T       : composite PSUM tile [128, 1024] per tkc: cols 0-511 =
#                    h0 (tile (0,0), contracts partitions 0-63), cols
#                    512-1023 = h1 (tile (64,0)) -> the two matmuls run
#                    CONCURRENTLY on disjoint PE row groups; ONE 1024-wide
#                    exp instruction serves both heads.
#   attn@v         : ot_h [65, 512] PSUM accumulators (1 bank each).
#   norm           : denom row DVE-copied out, reciprocal + gpsimd broadcast,
#                    one [128,512] DVE mul into on_sb.
#   out-proj       : y[t, :] = on^T @ woT per 128-token chunk, psum halves
#                    from the shared weave pool, bf16 staged, DMA'd out.

import numpy as np
import ml_dtypes

import concourse.bass as bass
import concourse.mybir as mybir
import concourse.tile as tile
from concourse import bacc

B, N, D, H = 2, 2048, 1024, 16
HD = 64
T = B * N                 # 4096 tokens
NCORES = 8
HPC = H // NCORES         # 2 heads per core
E = HPC * HD              # 128 per-core projection columns
KD = D // 128             # 8 contraction tiles for d
ROPE_BASE = 10000.0

BF = mybir.dt.bfloat16
F32 = mybir.dt.float32

TQB = 512                 # tq block (psum width per head)
NB = N // TQB             # 4 blocks per batch
NKC = N // 128            # 16 key chunks per batch
EW = 2 * TQB              # composite exp width (both heads)


def build_nc():
    nc = bacc.Bacc(trn_type="TRN2", target_bir_lowering=False, debug=False)

    xT = nc.dram_tensor("xT", [D, T], BF, kind="ExternalInput").ap()
    wqT = nc.dram_tensor("wqT", [D, E], BF, kind="ExternalInput").ap()
    wkT = nc.dram_tensor("wkT", [D, E], BF, kind="ExternalInput").ap()
    wvT = nc.dram_tensor("wvT", [D, E], BF, kind="ExternalInput").ap()
    woT = nc.dram_tensor("woT", [E, D], BF, kind="ExternalInput").ap()
    bcol = nc.dram_tensor("bcol", [E, 3], F32, kind="ExternalInput").ap()
    cosb = nc.dram_tensor("cosb", [E, N], BF, kind="ExternalInput").ap()
    sinb = nc.dram_tensor("sinb", [E, N], BF, kind="ExternalInput").ap()
    rotT = nc.dram_tensor("rotT", [E, E], BF, kind="ExternalInput").ap()
    idT = nc.dram_tensor("idT", [E, E], BF, kind="ExternalInput").ap()
    y = nc.dram_tensor("y", [T, D], BF, kind="ExternalOutput").ap()

    with tile.TileContext(nc) as tc:
        _build(tc, nc, xT, wqT, wkT, wvT, woT, bcol, cosb, sinb,
               rotT, idT, y)
    nc.compile()
    return nc


def _build(tc, nc, xT, wqT, wkT, wvT, woT, bcol, cosb, sinb,
           rotT, idT, y):
    with (
        tc.tile_pool(name="consts", bufs=1) as consts,
        tc.tile_pool(name="xbig", bufs=1) as xbig,
        tc.tile_pool(name="ebig", bufs=1) as ebig,
        tc.tile_pool(name="acts", bufs=1) as acts,
        tc.tile_pool(name="small", bufs=3) as small,
    ):
        # ---- constants / weights ----
        wq_sb = consts.tile([128, KD, E], BF, tag="wq")
        wk_sb = consts.tile([128, KD, E], BF, tag="wk")
        wv_sb = consts.tile([128, KD, E], BF, tag="wv")
        nc.sync.dma_start(out=wq_sb, in_=wqT.rearrange("(k p) e -> p k e", p=128))
        x_sb = xbig.tile([128, KD, T], BF, tag="big")
        xTr = xT.rearrange("(k p) t -> p k t", p=128)
        nc.sync.dma_start(out=x_sb[:, :, 0:512], in_=xTr[:, :, 0:512])
        nc.sync.dma_start(out=wk_sb, in_=wkT.rearrange("(k p) e -> p k e", p=128))
        nc.sync.dma_start(out=wv_sb, in_=wvT.rearrange("(k p) e -> p k e", p=128))
        nc.sync.dma_start(out=x_sb[:, :, 512:1024], in_=xTr[:, :, 512:1024])
        wo_sb = consts.tile([E, D], BF, tag="wo")
        nc.sync.dma_start(out=wo_sb, in_=woT)
        bcol_sb = consts.tile([E, 3], F32, tag="bcol")
        nc.sync.dma_start(out=bcol_sb, in_=bcol)
        cos_sb = consts.tile([E, N], BF, tag="cos")
        sin_sb = consts.tile([E, N], BF, tag="sin")
        nc.sync.dma_start(out=cos_sb, in_=cosb)
        nc.sync.dma_start(out=sin_sb, in_=sinb)
        rot_sb = consts.tile([E, E], BF, tag="rot")
        nc.sync.dma_start(out=rot_sb, in_=rotT)
        id_sb = consts.tile([E, E], BF, tag="idT")
        nc.sync.dma_start(out=id_sb, in_=idT)

        for ci in range(2, T // 512):
            nc.sync.dma_start(out=x_sb[:, :, ci * 512:(ci + 1) * 512],
                              in_=xTr[:, :, ci * 512:(ci + 1) * 512])

        # ---- persistent activations ----
        q2 = acts.tile([E, T], BF, tag="q2")
        k2 = acts.tile([E, T], BF, tag="k2")
        v_sb = acts.tile([128, T // 128, HPC, HD + 1], BF, tag="v_sb")
        on_sb = acts.tile([E, B, N], BF, tag="on_sb")

        nc.vector.memset(v_sb[:, :, :, HD:HD + 1], 1.0)

        # ---------- projection emitters (used in phase 1a and the weave) ----
        def proj_qk_slice(ps_pool, psr_pool, ci, dst, w, bc, granules=None):
            # q or k projection for 512-token slice ci, rope'd in place.
            # When `granules` is a list, work is appended as deferred items.
            # PSUM tiles are allocated inside the granule bodies so pool
            # slot rotation matches emission order.
            sl = slice(ci * 512, (ci + 1) * 512)
            npos = (ci * 512) % N
            tsl = slice(npos, npos + 512)
            cell = {}

            def part_mm(k):
                def emit():
                    if k == 0:
                        cell["ps"] = ps_pool.tile([128, 512], F32,
                                                  tag="ps_qk",
                                                  name=f"ps_{bc}_{ci}")
                    ps = cell["ps"]
                    nc.tensor.matmul(ps, w[:, k, :], x_sb[:, k, sl],
                                     start=(k == 0), stop=(k == KD - 1))
                    if k == KD - 1:
                        nc.vector.tensor_scalar_add(
                            dst[:, sl], ps, bcol_sb[:, bc:bc + 1])
                return emit

            def part_rope():
                psr = psr_pool.tile([128, 512], F32, tag="ps_qk",
                                    name=f"psr_{bc}_{ci}")
                nc.tensor.matmul(psr, rot_sb, dst[:, sl], start=True, stop=True)
                t1 = small.tile([128, 512], BF, tag="rope_t1")
                nc.vector.tensor_mul(t1, dst[:, sl], cos_sb[:, tsl])
                t2 = small.tile([128, 512], BF, tag="rope_t2")
                nc.vector.tensor_mul(t2, psr, sin_sb[:, tsl])
                nc.vector.tensor_add(dst[:, sl], t1, t2)

            if granules is None:
                for k in range(KD):
                    part_mm(k)()
                part_rope()
            else:
                granules.extend([part_mm(k) for k in range(KD)])
                granules.append(part_rope)

        def proj_v_slice(ps_pool, pst_pool, ci):
            sl = slice(ci * 512, (ci + 1) * 512)
            psv = ps_pool.tile([128, 512], F32, tag="ps_qk", name=f"psv_{ci}")
            for k in range(KD):
                nc.tensor.matmul(psv, wv_sb[:, k, :], x_sb[:, k, sl],
                                 start=(k == 0), stop=(k == KD - 1))
            vts = small.tile([128, 512], BF, tag="vts")
            nc.vector.tensor_scalar_add(vts, psv, bcol_sb[:, 2:3])
            for s in range(4):
                cv = ci * 4 + s
                pst = pst_pool.tile([128, 128], BF, tag="ps_t",
                                    name=f"pst_{ci}_{s}")
                nc.tensor.transpose(pst, vts[:, s * 128:(s + 1) * 128], id_sb)
                nc.vector.tensor_copy(v_sb[:, cv, :, 0:HD], pst)

        # ================= phase 1a =================
        # b=0 projections (slices 0-3) fully, plus v for b=1 (slices 4-7,
        # transposes need their own psum which the drum can't spare).
        with (
            tc.tile_pool(name="ps_qk", bufs=2, space="PSUM") as ps_qk,
            tc.tile_pool(name="ps_r", bufs=2, space="PSUM") as ps_r,
            tc.tile_pool(name="ps_t", bufs=2, space="PSUM") as ps_t,
        ):
            for ci in range(4):
                proj_qk_slice(ps_qk, ps_r, ci, k2, wk_sb, 1)
                proj_v_slice(ps_qk, ps_t, ci)
                proj_qk_slice(ps_qk, ps_r, ci, q2, wq_sb, 0)
            for ci in range(4, 8):
                proj_v_slice(ps_qk, ps_t, ci)

        # ========= phase 2: attention drum + woven work =========
        with (
            tc.tile_pool(name="ps_sc", bufs=2, space="PSUM") as ps_sc,
            tc.tile_pool(name="ps_o", bufs=2, space="PSUM") as ps_o,
            tc.tile_pool(name="ps_w", bufs=2, space="PSUM") as ps_w,
        ):
            critical = []   # b=1 q/k projection granules (must finish in b0)
            pending = []    # norm / out-proj items (may spill)
            _state = {"drain": False}

            # enqueue the b=1 q/k projection granules
            for ci in range(4, 8):
                proj_qk_slice(ps_w, ps_w, ci, k2, wk_sb, 1, granules=critical)
            for ci in range(4, 8):
                proj_qk_slice(ps_w, ps_w, ci, q2, wq_sb, 0, granules=critical)

            def norm_item(b, qb, ou2f, rss):
                def emit():
                    for h in range(HPC):
                        rc = small.tile([1, TQB], F32, tag="recip", bufs=2,
                                        name=f"rc_{b}_{qb}_{h}")
                        nc.vector.reciprocal_approx_fast(out=rc, in_=rss[h])
                        rbs = small.tile([128, TQB], F32, tag="recipb",
                                         bufs=2, name=f"rbs_{b}_{qb}_{h}")
                        nc.gpsimd.partition_broadcast(
                            out_ap=rbs, in_ap=rc, channels=128)
                        hsl = slice(h * HD, (h + 1) * HD)
                        nc.vector.tensor_mul(
                            on_sb[hsl, b, qb * TQB:(qb + 1) * TQB],
                            ou2f[hsl, :], rbs[hsl, :])
                    for ci in range(qb * 4, qb * 4 + 4):
                        ycell = {}
                        for eo in range(D // 512):
                            pending.append(y_item(b, ci, eo, ycell))
                return emit

            def y_item(b, ci, eo, cell):
                # one 512-col half of the out-projection for 128 tokens
                def emit():
                    if eo == 0:
                        cell["ysb"] = small.tile([128, D], BF, tag="ysb",
                                                 bufs=4,
                                                 name=f"ysb_{b}_{ci}")
                    ysb = cell["ysb"]
                    psy = ps_w.tile([128, 512], F32, tag="ps_qk",
                                    name=f"psy_{b}_{ci}_{eo}")
                    nc.tensor.matmul(
                        psy,
                        on_sb[:, b, ci * 128:(ci + 1) * 128],
                        wo_sb[:, eo * 512:(eo + 1) * 512],
                        start=True, stop=True)
                    if _state["drain"] and (ci + eo) % 2 == 1:
                        nc.scalar.copy(ysb[:, eo * 512:(eo + 1) * 512], psy)
                    else:
                        nc.vector.tensor_copy(
                            ysb[:, eo * 512:(eo + 1) * 512], psy)
                    if eo == D // 512 - 1:
                        nc.sync.dma_start(
                            out=y[b * N + ci * 128: b * N + (ci + 1) * 128, :],
                            in_=ysb)
                return emit

            def pop_item():
                if critical:
                    critical.pop(0)()
                elif pending:
                    pending.pop(0)()

            # ---- flat global software pipeline over (b, qb, tkc) ----
            # Per iteration i: emit scores(i+1) [one period AHEAD of the
            # exp that consumes it, so ACT never waits on the PE FIFO],
            # exp(i), attn(i-1), block-end evacuation, then one deferred
            # item. This keeps the exp drum gap-free across block
            # boundaries too.
            triples = [(b, qb, tkc)
                       for b in range(B) for qb in range(NB)
                       for tkc in range(NKC)]
            NTR = len(triples)
            blk_state = {}
            sc_of = {}

            def emit_scores(i):
                b, qb, tkc = triples[i]
                tq0 = b * N + qb * TQB
                sc = ps_sc.tile([128, EW], F32, tag="ps_sc",
                                name=f"sc_{b}_{qb}_{tkc}")
                sc_of[i] = sc
                # h0 -> cols 0-511 (PE tile (0,0)), h1 -> cols 512-1023
                # (PE tile (64,0)): concurrent matmuls.
                for h in range(HPC):
                    nc.tensor.matmul(
                        sc[:, h * TQB:(h + 1) * TQB],
                        k2[h * HD:(h + 1) * HD,
                           b * N + tkc * 128: b * N + (tkc + 1) * 128],
                        q2[h * HD:(h + 1) * HD, tq0: tq0 + TQB],
                        start=True, stop=True)

            NEARLY = 4   # first chunks use a decoupled double-buffered tile

            def exp_slot(st, b, qb, tkc):
                if tkc < NEARLY:
                    if "exp0" not in st:
                        st["exp0"] = small.tile([128, NEARLY, EW], BF,
                                                tag="exp0", bufs=2,
                                                name=f"exp0_{b}_{qb}")
                    return st["exp0"][:, tkc, :]
                if "exp_t" not in st:
                    st["exp_t"] = ebig.tile([128, NKC, EW], BF, tag="exp",
                                            name=f"exp_{b}_{qb}")
                return st["exp_t"][:, tkc, :]

            def emit_exp(i):
                b, qb, tkc = triples[i]
                st = blk_state.setdefault((b, qb), {})
                nc.scalar.activation(
                    out=exp_slot(st, b, qb, tkc), in_=sc_of.pop(i),
                    func=mybir.ActivationFunctionType.Exp,
                    scale=float(HD) ** -0.5)

            def emit_attn(i):
                b, qb, tkc = triples[i]
                st = blk_state[(b, qb)]
                if tkc == 0:
                    st["ots"] = [ps_o.tile([HD + 1, TQB], F32, tag="ps_o",
                                           name=f"ot_{b}_{qb}_{h}")
                                 for h in range(HPC)]
                src = (st["exp0"][:, tkc, :] if tkc < NEARLY
                       else st["exp_t"][:, tkc, :])
                for h in range(HPC):
                    nc.tensor.matmul(
                        st["ots"][h], v_sb[:, b * NKC + tkc, h, :],
                        src[:, h * TQB:(h + 1) * TQB],
                        start=(tkc == 0), stop=(tkc == NKC - 1))
                if tkc == NKC - 1:
                    emit_evac(b, qb, st)

            def emit_evac(b, qb, st):
                # fast PSUM evacuation (DVE); recip chain deferred
                ots = st["ots"]
                ou2f = small.tile([128, TQB], F32, tag="ou", bufs=2,
                                  name=f"ou_{b}_{qb}")
                nc.vector.tensor_copy(ou2f[0:HD, :], ots[0][0:HD, :])
                nc.vector.tensor_copy(ou2f[HD:2 * HD, :], ots[1][0:HD, :])
                rss = []
                for h in range(HPC):
                    rs = small.tile([1, TQB], F32, tag="rs", bufs=4,
                                    name=f"rs_{b}_{qb}_{h}")
                    nc.vector.tensor_copy(rs, ots[h][HD:HD + 1, :])
                    rss.append(rs)
                pending.append(norm_item(b, qb, ou2f, rss))

            for i in range(NTR):
                if i == 0:
                    emit_scores(0)
                if i + 1 < NTR:
                    emit_scores(i + 1)
                emit_exp(i)
                if i > 0:
                    emit_attn(i - 1)
                if i >= 1:
                    pop_item()
                    # drain the backlog harder when it exceeds remaining slots
                    if len(critical) + len(pending) > NTR - i:
                        pop_item()
            emit_attn(NTR - 1)

            _state["drain"] = True
            while critical:
                critical.pop(0)()
            while pending:
                pending.pop(0)()


def _host_inputs(x, Wq, Wk, Wv, Wo, bq, bk, bv, bo):
    """Build the 8 per-core input maps (host-side sharding + layout prep)."""
    bf16 = ml_dtypes.bfloat16
    xTh = np.ascontiguousarray(x.reshape(T, D).T).astype(bf16)

    i = (np.arange(E) % HD) % (HD // 2)
    inv_freq = ROPE_BASE ** (-2.0 * i / HD)  # [E]
    ang = np.arange(N)[None, :] * inv_freq[:, None]          # [E, N]
    cosb = np.cos(ang).astype(bf16)
    sinb = np.sin(ang).astype(bf16)

    P = np.zeros((E, E), dtype=np.float32)
    for h in range(HPC):
        for j in range(HD // 2):
            P[h * HD + j, h * HD + j + HD // 2] = -1.0
            P[h * HD + j + HD // 2, h * HD + j] = 1.0
    rotT = np.ascontiguousarray(P.T).astype(bf16)
    idT = np.eye(E, dtype=np.float32).astype(bf16)

    in_maps = []
    for c in range(NCORES):
        sl = slice(c * E, (c + 1) * E)
        in_maps.append({
            "xT": xTh,
            "wqT": np.ascontiguousarray(Wq[sl, :].T).astype(bf16),
            "wkT": np.ascontiguousarray(Wk[sl, :].T).astype(bf16),
            "wvT": np.ascontiguousarray(Wv[sl, :].T).astype(bf16),
            "woT": np.ascontiguousarray(Wo[:, sl].T).astype(bf16),
            "bcol": np.stack([bq[sl], bk[sl], bv[sl]], axis=1)
                .astype(np.float32),
            "cosb": cosb,
            "sinb": sinb,
            "rotT": rotT,
            "idT": idT,
        })
    return in_maps


_NC = None


def kernel(x, Wq, Wk, Wv, Wo, bq, bk, bv, bo):
    from concourse.bass_utils import run_bass_kernel_spmd

    global _NC
    if _NC is None:
        _NC = build_nc()
    bo = np.asarray(bo, dtype=np.float32)
    in_maps = _host_inputs(np.asarray(x, dtype=np.float32),
                           np.asarray(Wq, dtype=np.float32),
                           np.asarray(Wk, dtype=np.float32),
                           np.asarray(Wv, dtype=np.float32),
                           np.asarray(Wo, dtype=np.float32),
                           np.asarray(bq, dtype=np.float32),
                           np.asarray(bk, dtype=np.float32),
                           np.asarray(bv, dtype=np.float32),
                           bo)
    res = run_bass_kernel_spmd(_NC, in_maps, core_ids=list(range(NCORES)))
    out = np.zeros((T, D), dtype=np.float32)
    for r in res.results:
        out += np.asarray(r["y"], dtype=np.float32)
    out += bo[None, :]
    return out.reshape(B, N, D)
